# revision 1
# baseline (speedup 1.0000x reference)
"""Trainium2 Bass kernel for CliffordAdaptiveGraphAttention.

Math restructuring (validated numerically against the reference):
  * The "clifford enhancement" term `enh` is added uniformly to all 8 head
    scores, so it cancels in softmax -> only the *symmetrized* geometric
    product matters: ip = 0.5*(gp(sa,da) + gp(da,sa)) = sa^T Gsym da.
  * Work in a permuted Clifford basis (grade-ordered) so the vector
    components sit in contiguous slots 1..4.
  * mp2 columns are permuted into an L^T layout so the Cholesky factor comes
    out of the matmul directly; softplus on the diagonal via a degree-6
    polynomial (Horner on gpsimd).
  * metric @ v evaluated as L (L^T v) per edge batch-major on gpsimd.

Performance structure (v2):
  * Inputs are converted to bf16 host-side and DMA'd feature-major via the
    XBAR transposing DMA (dma_start_transpose) - no PE input transposes and
    no PSUM->SBUF input copies; input HBM traffic halved.
  * All remaining PE transposes stream a bf16 identity (the identity is the
    moving operand, so transpose cost is 1 cyc/row instead of 2 for f32).
  * Elementwise plumbing tensors are bf16 where precision allows, which
    unlocks the DVE 2x/4x packed modes and halves SBUF footprint.
  * mt/sct/dct heads share one PSUM tile ([48,CH]) and one SBUF tile, so the
    batch-major transposes run 4x[48,128] instead of 12x[16,128].
  * P = sa (x) da is produced directly in bf16; its transposes and the
    Gsym contraction all run bf16.
"""

import numpy as np
import ml_dtypes

import bass_rust as _bass_rust
import concourse.bass as bass
import concourse.mybir as mybir
from concourse.alu_op_type import AluOpType
from concourse.bass import ts
from concourse.tile import TileContext
from concourse.bass_utils import run_bass_kernel_spmd

F32 = mybir.dt.float32
FR = mybir.dt.float32r
BF = mybir.dt.bfloat16
AF = mybir.ActivationFunctionType
BF_NP = ml_dtypes.bfloat16

N_CORES = 8
B_FULL = 262144
D = 128
CH = 512          # edges per chunk
SUB = CH // 128   # 128-row subtiles per chunk

# grade-ordered Clifford basis permutation for Cl(4,0)
PERM = np.array([0, 1, 2, 4, 8, 3, 5, 6, 9, 10, 12, 7, 11, 13, 14, 15])

# ---- f32 consts layout (columns of the packed [128, NCOLS] array) ----
OFF_WI2 = 0
OFF_WMP1 = 128
OFF_WMP2 = 256
OFF_ID = 272      # f32 identity (used via fp32r bitcast for transposes)
OFF_BIAS = 400
# bias cols: 2 im1, 3 im2, 4 mp1, 5 q, 8 outc, 9 mp2pp(16), 10 ntc2(48: tanh
#            bias rows 0:16 then ntc bias twice)
OFF_SPC = OFF_BIAS + 11   # 7 softplus polynomial coeff columns (c0..c6)
NCOLS = OFF_SPC + 7

# ---- bf16 consts layout (columns of the packed [128, NBCOLS] array) ----
BO_ID = 0         # bf16 identity [128,128]
BO_WNP = 128
BO_WEP = 256
BO_WQ = 384
BO_WK = 512
BO_WV = 640
BO_WOUT = 768
BO_EXP8 = 896     # [8 -> 128] head-broadcast lhsT
BO_WNTC = 1024    # 16
BO_GA = 1040      # 16
BO_GB = 1056      # 16
BO_WCTO = 1072    # 128 (rows 0:16)
BO_BLK = 1200     # 8: per-head reduction lhsT
BO_ONES = 1208    # 8 cols of ones
NBCOLS = 1216


def _geo_table():
    N = 4
    BASIS = 16
    geo = np.zeros((BASIS, BASIS, BASIS), np.float64)
    for i in range(BASIS):
        bi = [(i >> (N - 1 - k)) & 1 for k in range(N)]
        for j in range(BASIS):
            bj = [(j >> (N - 1 - k)) & 1 for k in range(N)]
            sign = 1.0
            for k in range(N):
                if bj[k] and sum(bi[:k]) % 2 == 1:
                    sign = -sign
            geo[i, j, i ^ j] = sign
    return geo


def _pack_consts(w):
    """Build the [128, NCOLS] fp32 and [128, NBCOLS] bf16 constants arrays."""
    C = np.zeros((128, NCOLS), np.float32)
    CB = np.zeros((128, NBCOLS), np.float32)

    CB[:, BO_ID:BO_ID + 128] = np.eye(128, dtype=np.float32)
    # first two linear layers fused: relu(src@A + edge@B + b') with
    # A = np_W @ im1_W[:128], B = ep_W @ im1_W[128:]
    im1a = w["im1_W"][:128].astype(np.float64)
    im1b = w["im1_W"][128:].astype(np.float64)
    CB[:, BO_WNP:BO_WNP + 128] = (w["np_W"].astype(np.float64) @ im1a)
    CB[:, BO_WEP:BO_WEP + 128] = (w["ep_W"].astype(np.float64) @ im1b)
    CB[:, BO_WQ:BO_WQ + 128] = w["q_W"]
    CB[:, BO_WK:BO_WK + 128] = w["k_W"]
    CB[:, BO_WV:BO_WV + 128] = w["v_W"]
    CB[:, BO_WOUT:BO_WOUT + 128] = w["out_W"]

    C[:, OFF_WI2:OFF_WI2 + 128] = w["im2_W"]
    C[:, OFF_WMP1:OFF_WMP1 + 128] = w["mp1_W"]
    C[:, OFF_ID:OFF_ID + 128] = np.eye(128, dtype=np.float32)

    # mp2 in L^T ("Lt") layout: col j*4+i <- m[i,j] for i>=j, else zero
    W16 = np.zeros((128, 16), np.float32)
    b16 = np.zeros(16, np.float32)
    for j in range(4):
        for i in range(4):
            if i >= j:
                W16[:, j * 4 + i] = w["mp2_W"][:, i * 4 + j]
                b16[j * 4 + i] = w["mp2_b"][i * 4 + j]
    C[:, OFF_WMP2:OFF_WMP2 + 16] = W16

    ntc_Wp = w["ntc_W"][:, PERM]
    ntc_bp = w["ntc_b"][PERM]
    CB[:, BO_WNTC:BO_WNTC + 16] = ntc_Wp

    geo = _geo_table()
    gsym = 0.5 * (geo + np.swapaxes(geo, 0, 1))
    gsym_p = gsym[np.ix_(PERM, PERM, PERM)].reshape(256, 16).astype(np.float32)
    CB[:, BO_GA:BO_GA + 16] = gsym_p[:128]
    CB[:, BO_GB:BO_GB + 16] = gsym_p[128:]

    CB[:16, BO_WCTO:BO_WCTO + 128] = w["cto_W"][PERM, :]

    # expand8[h, hd] = 1 if hd//16 == h  (lhsT for head-broadcast matmul)
    exp8 = np.zeros((8, 128), np.float32)
    for h in range(8):
        exp8[h, h * 16:(h + 1) * 16] = 1.0
    CB[:8, BO_EXP8:BO_EXP8 + 128] = exp8
    CB[:, BO_BLK:BO_BLK + 8] = exp8.T
    CB[:, BO_ONES:BO_ONES + 8] = 1.0

    C[:, OFF_BIAS + 2] = (w["np_b"].astype(np.float64) @ im1a
                          + w["ep_b"].astype(np.float64) @ im1b
                          + w["im1_b"])
    C[:, OFF_BIAS + 3] = w["im2_b"]
    C[:, OFF_BIAS + 4] = w["mp1_b"]
    C[:, OFF_BIAS + 5] = w["q_b"]
    C[:, OFF_BIAS + 6] = w["k_b"]
    C[:, OFF_BIAS + 7] = w["v_b"]
    C[:, OFF_BIAS + 8] = w["out_b"] + w["cto_b"]
    C[:16, OFF_BIAS + 9] = b16
    # bias for the merged [mt@0|sct@32|dct@64] tile: ntc bias at rows 32:48
    # and 64:80
    C[32:48, OFF_BIAS + 10] = ntc_bp
    C[64:80, OFF_BIAS + 10] = ntc_bp

    # softplus-on-[-1,1] polynomial coefficients, replicated per partition
    xs = np.linspace(-1.0, 1.0, 20001)
    cheb = np.polynomial.chebyshev.Chebyshev.fit(xs, np.log1p(np.exp(xs)), 6)
    spc = cheb.convert(kind=np.polynomial.Polynomial).coef
    for k in range(7):
        C[:, OFF_SPC + k] = spc[k]
    return C, CB.astype(BF_NP)


def build_kernel(n_edges, stage=6, reps=1):
    """Build the per-core Bass program: n_edges rows -> n_edges rows.

    Software-pipelined: chunk c's serial tail (metric/P/output) is emitted
    around chunk c+1's head (loads/attention/MLP).
    """
    assert n_edges % CH == 0
    n_chunks = n_edges // CH

    nc = bass.Bass()
    src_d = nc.dram_tensor("src", [n_edges, D], BF, kind="ExternalInput")
    dst_d = nc.dram_tensor("dst", [n_edges, D], BF, kind="ExternalInput")
    edg_d = nc.dram_tensor("edg", [n_edges, D], BF, kind="ExternalInput")
    cst_d = nc.dram_tensor("consts", [128, NCOLS], F32, kind="ExternalInput")
    cbf_d = nc.dram_tensor("cbf", [128, NBCOLS], BF, kind="ExternalInput")
    out_d = nc.dram_tensor("out", [n_edges, D], F32, kind="ExternalOutput")

    with TileContext(nc) as tc, nc.allow_low_precision(
            reason="bf16/fp32r matmul operand rounding"):
        with (
            tc.tile_pool(name="cst", bufs=1) as cstp,
            tc.tile_pool(name="inn", bufs=3) as innp,
            tc.tile_pool(name="act", bufs=3) as actp,
            tc.tile_pool(name="big", bufs=3) as bigp,
            tc.tile_pool(name="sml", bufs=3) as smlp,
            tc.tile_pool(name="psm", bufs=3, space="PSUM") as psm,
            tc.tile_pool(name="pss", bufs=2, space="PSUM") as pss,
            tc.tile_pool(name="ptp", bufs=1, space="PSUM") as ptp,
        ):
            cst = cstp.tile([128, NCOLS], F32)
            nc.sync.dma_start(out=cst[:], in_=cst_d[:])
            cbf = cstp.tile([128, NBCOLS], BF)
            nc.sync.dma_start(out=cbf[:], in_=cbf_d[:])

            # one-time rounding copy of interior fp32 weights to fp32r
            cstr = cstp.tile([128, OFF_BIAS], FR, tag="cstr")
            nc.vector.tensor_copy(cstr[:], cst[:, 0:OFF_BIAS])

            identb = cbf[:, BO_ID:BO_ID + 128]
            identr = cstr[:, OFF_ID:OFF_ID + 128]

            def WB(off, n=128, p=128):
                return cbf[:p, off:off + n]

            def WR(off, n=128, p=128):
                return cstr[:p, off:off + n]

            def bias(j, p=128, r0=0):
                return cst[r0:r0 + p, OFF_BIAS + j:OFF_BIAS + j + 1]

            def emit_head(c):
                row0 = c * CH
                st = {}
                fm = {}
                for name, dram in (("src", src_d), ("dst", dst_d),
                                   ("edg", edg_d)):
                    t = innp.tile([128, CH], BF, tag=f"in_{name}")
                    nc.sync.dma_start_transpose(
                        t[:], dram[row0:row0 + CH, :])
                    fm[name] = t

                # attention matmuls (independent of the MLP chain)
                pq = psm.tile([128, CH], F32, tag="mm")
                nc.tensor.matmul(pq[:], WB(BO_WQ), fm["src"][:], start=True,
                                 stop=True)
                pk = psm.tile([128, CH], F32, tag="mm")
                nc.tensor.matmul(pk[:], WB(BO_WK), fm["dst"][:], start=True,
                                 stop=True)
                ksb = smlp.tile([128, CH], BF, tag="ksb")
                nc.scalar.activation(ksb[:], pk[:], AF.Identity, bias=bias(6))

                # metric MLP layer 1 (np/ep fused into im1)
                ps1 = psm.tile([128, CH], F32, tag="mm")
                nc.tensor.matmul(ps1[:], WB(BO_WNP), fm["src"][:], start=True,
                                 stop=False)
                nc.tensor.matmul(ps1[:], WB(BO_WEP), fm["edg"][:], start=False,
                                 stop=True)
                i1 = actp.tile([128, CH], FR, tag="i1")
                nc.scalar.activation(i1[:], ps1[:], AF.Relu, bias=bias(2))

                pv = psm.tile([128, CH], F32, tag="mm")
                nc.tensor.matmul(pv[:], WB(BO_WV), fm["dst"][:], start=True,
                                 stop=True)
                vsb = smlp.tile([128, CH], BF, tag="vsb")
                nc.scalar.activation(vsb[:], pv[:], AF.Identity, bias=bias(7))
                prod = bigp.tile([128, CH], BF, tag="prod")
                nc.vector.scalar_tensor_tensor(
                    prod[:], pq[:], bias(5), ksb[:],
                    op0=AluOpType.add, op1=AluOpType.mult)
                psc = pss.tile([8, CH], F32, tag="sm")
                nc.tensor.matmul(psc[:], WB(BO_BLK, n=8), prod[:],
                                 start=True, stop=True)
                e = smlp.tile([8, CH], BF, tag="e")
                nc.scalar.activation(e[:], psc[:], AF.Exp, scale=0.25)

                ps2 = psm.tile([128, CH], F32, tag="mm")
                nc.tensor.matmul(ps2[:], WR(OFF_WI2), i1[:], start=True,
                                 stop=True)
                i2 = actp.tile([128, CH], FR, tag="i2")
                nc.scalar.activation(i2[:], ps2[:], AF.Relu, bias=bias(3))

                pden = pss.tile([1, CH], F32, tag="sm")
                nc.tensor.matmul(pden[:], WB(BO_ONES, n=1, p=8), e[:],
                                 start=True, stop=True)
                rden = smlp.tile([1, CH], BF, tag="rden")
                nc.vector.reciprocal(rden[:], pden[:])
                prd8 = pss.tile([8, CH], F32, tag="sm")
                nc.tensor.matmul(prd8[:], WB(BO_ONES, n=8, p=1), rden[:],
                                 start=True, stop=True)
                wsb = smlp.tile([8, CH], BF, tag="w")
                nc.vector.tensor_mul(wsb[:], e[:], prd8[:])

                ps3 = psm.tile([128, CH], F32, tag="mm")
                nc.tensor.matmul(ps3[:], WR(OFF_WMP1), i2[:], start=True,
                                 stop=True)
                m1 = actp.tile([128, CH], FR, tag="m1")
                nc.scalar.activation(m1[:], ps3[:], AF.Relu, bias=bias(4))

                pwr = psm.tile([128, CH], F32, tag="mm")
                nc.tensor.matmul(pwr[:], WB(BO_EXP8, p=8), wsb[:],
                                 start=True, stop=True)
                att = bigp.tile([128, CH], BF, tag="att")
                nc.vector.tensor_mul(att[:], pwr[:], vsb[:])
                st["att"] = att

                # mp2 + ntc heads; matmul outs must sit at PSUM base 0/32/64
                pmt = pss.tile([16, CH], F32, tag="sm")
                nc.tensor.matmul(pmt[:], WR(OFF_WMP2, n=16), m1[:],
                                 start=True, stop=True)
                pnt = pss.tile([48, CH], F32, tag="pnt", bufs=1)
                nc.tensor.matmul(pnt[0:16, :], WB(BO_WNTC, n=16),
                                 fm["src"][:], start=True, stop=True)
                nc.tensor.matmul(pnt[32:48, :], WB(BO_WNTC, n=16),
                                 fm["dst"][:], start=True, stop=True)
                # merged [mt@0 | sct@32 | dct@64] tile (engine writes need
                # 32-aligned partition bases) so the batch-major transpose
                # runs 4x[96,128] instead of 12x[16,128]; the pad segments
                # are never read downstream.
                msd = smlp.tile([96, CH], FR, tag="msd")
                nc.scalar.activation(msd[0:16, :], pmt[:], AF.Tanh,
                                     bias=bias(9, p=16))
                nc.scalar.activation(msd[32:48, :], pnt[0:16, :], AF.Identity,
                                     bias=bias(10, p=16, r0=32))
                nc.vector.tensor_scalar(msd[64:80, :], pnt[32:48, :],
                                        bias(10, p=16, r0=64), 0.0,
                                        op0=AluOpType.add,
                                        op1=AluOpType.bypass)
                st["msd"] = msd
                return st

            def emit_tail1(c, st):
                msd = st["msd"]
                bm = bigp.tile([128, SUB, 96], F32, tag="bm")
                for n in range(SUB):
                    pbm = ptp.tile([128, 96], FR, tag="tpo")
                    nc.tensor.transpose(pbm[:], msd[:, ts(n, 128)],
                                        identr[:96, :96])
                    nc.vector.tensor_copy(bm[:, n, :], pbm[:].bitcast(F32))

                # softplus on the tanh-bounded diagonal via a degree-6
                # polynomial (Horner, gpsimd tensor_tensor only).
                dg = bm[:, :, 0:16:5]

                def spcb(k):
                    return cst[:, OFF_SPC + k:OFF_SPC + k + 1] \
                        .unsqueeze(1).broadcast_to((128, SUB, 4))

                u = bigp.tile([128, SUB, 4], F32, tag="spt")
                nc.gpsimd.tensor_tensor(u[:], dg, spcb(6), op=AluOpType.mult)
                for k in (5, 4, 3, 2, 1):
                    nc.gpsimd.tensor_tensor(u[:], u[:], spcb(k),
                                            op=AluOpType.add)
                    nc.gpsimd.tensor_tensor(u[:], u[:], dg,
                                            op=AluOpType.mult)
                nc.gpsimd.tensor_tensor(dg, u[:], spcb(0), op=AluOpType.add)

                # adaptive metric: v <- L (L^T v), in place
                lt = bm[:, :, 0:16].rearrange("p n (j i) -> p n j i", j=4)
                for off in (32, 64):
                    v = bm[:, :, off + 1:off + 5]
                    tmp1 = bigp.tile([128, SUB, 4, 4], F32, tag="tmp1")
                    nc.gpsimd.tensor_tensor(
                        tmp1[:], lt,
                        v.unsqueeze(2).broadcast_to((128, SUB, 4, 4)),
                        op=AluOpType.mult)
                    t4 = bigp.tile([128, SUB, 4], F32, tag="t4")
                    nc.vector.reduce_sum(t4[:], tmp1[:],
                                         axis=mybir.AxisListType.X)
                    tmp2 = bigp.tile([128, SUB, 4, 4], F32, tag="tmp1")
                    nc.gpsimd.tensor_tensor(
                        tmp2[:], lt,
                        t4[:].unsqueeze(3).broadcast_to((128, SUB, 4, 4)),
                        op=AluOpType.mult)
                    u4 = bigp.tile([128, SUB, 4], F32, tag="t4")
                    nc.vector.reduce_sum(u4[:], tmp2[:].transpose([0, 1, 3, 2]),
                                         axis=mybir.AxisListType.X)
                    nc.gpsimd.tensor_copy(v, u4[:])

                # P = sa (x) da (batch-major, bf16 out)
                pP = bigp.tile([128, SUB, 256], BF, tag="pP")
                sa = bm[:, :, 32:48]
                da = bm[:, :, 64:80]
                nc.gpsimd.tensor_tensor(
                    pP[:],
                    sa.unsqueeze(3).broadcast_to((128, SUB, 16, 16)),
                    da.unsqueeze(2).broadcast_to((128, SUB, 16, 16)),
                    op=AluOpType.mult)
                st["pP"] = pP

            def emit_tail2(c, st):
                row0 = c * CH
                pP, att = st["pP"], st["att"]
                pip = pss.tile([16, CH], F32, tag="sm")

                pfa = ptp.tile([128, CH], BF, tag="tpa")
                for n in range(SUB):
                    nc.tensor.transpose(pfa[:, ts(n, 128)], pP[:, n, 0:128],
                                        identb)
                Pa = bigp.tile([128, CH], BF, tag="Pa")
                nc.vector.tensor_copy(Pa[:], pfa[:])
                nc.tensor.matmul(pip[:], WB(BO_GA, n=16), Pa[:],
                                 start=True, stop=False)

                pfb = ptp.tile([128, CH], BF, tag="tpa")
                for n in range(SUB):
                    nc.tensor.transpose(pfb[:, ts(n, 128)], pP[:, n, 128:256],
                                        identb)
                Pb = bigp.tile([128, CH], BF, tag="Pb")
                nc.vector.tensor_copy(Pb[:], pfb[:])
                nc.tensor.matmul(pip[:], WB(BO_GB, n=16), Pb[:],
                                 start=False, stop=True)
                ip = smlp.tile([16, CH], BF, tag="ip")
                nc.scalar.copy(ip[:], pip[:])

                po = psm.tile([128, CH], F32, tag="mm")
                nc.tensor.matmul(po[:], WB(BO_WOUT), att[:], start=True,
                                 stop=False)
                nc.tensor.matmul(po[:], WB(BO_WCTO, p=16), ip[:],
                                 start=False, stop=True)
                of = bigp.tile([128, CH], FR, tag="of")
                nc.scalar.activation(of[:], po[:], AF.Identity, bias=bias(8))

                pot = ptp.tile([128, SUB, 128], FR, tag="tpo")
                for n in range(SUB):
                    nc.tensor.transpose(pot[:, n, :], of[:, ts(n, 128)],
                                        identr)
                ot = bigp.tile([128, SUB, 128], F32, tag="ot")
                nc.vector.tensor_copy(ot[:], pot[:].bitcast(F32))
                nc.sync.dma_start(
                    out=out_d[row0:row0 + CH, :].rearrange(
                        "(n p) f -> p n f", p=128),
                    in_=ot[:])

            prev = None
            prev_c = None
            for i in range(n_chunks * reps):
                c = i % n_chunks
                st = emit_head(c)
                emit_tail1(c, st)
                if prev is not None:
                    emit_tail2(prev_c, prev)
                prev, prev_c = st, c
            emit_tail2(prev_c, prev)

    # TRN2 allows at most one semaphore wait per engine instruction; split
    # multi-wait instructions through event semaphores (same pass bacc runs).
    _bass_rust.generate_event_semaphores(nc)
    return nc


_CACHE = {}


def _get_kernel(n_edges, stage=6):
    key = (n_edges, stage)
    if key not in _CACHE:
        _CACHE[key] = build_kernel(n_edges, stage)
    return _CACHE[key]


def kernel(**inputs):
    w = {k: np.asarray(v, np.float32) for k, v in inputs.items()
         if k not in ("src_nodes", "dst_nodes", "edge_features", "edge_indices")}
    src = np.ascontiguousarray(
        np.asarray(inputs["src_nodes"], np.float32).astype(BF_NP))
    dst = np.ascontiguousarray(
        np.asarray(inputs["dst_nodes"], np.float32).astype(BF_NP))
    edg = np.ascontiguousarray(
        np.asarray(inputs["edge_features"], np.float32).astype(BF_NP))

    B = src.shape[0]
    assert B % N_CORES == 0
    bc = B // N_CORES

    consts, cbf = _pack_consts(w)
    nc = _get_kernel(bc)

    in_maps = []
    for i in range(N_CORES):
        sl = slice(i * bc, (i + 1) * bc)
        in_maps.append({
            "src": src[sl], "dst": dst[sl], "edg": edg[sl],
            "consts": consts, "cbf": cbf,
        })
    res = run_bass_kernel_spmd(nc, in_maps, list(range(N_CORES)))
    return np.concatenate([res.results[i]["out"] for i in range(N_CORES)], axis=0)



# revision 16
# speedup vs baseline: 1.2708x; 1.2708x over previous
"""Trainium2 Bass kernel for CliffordAdaptiveGraphAttention.

Math restructuring (validated numerically against the reference):
  * The "clifford enhancement" term `enh` is added uniformly to all 8 head
    scores, so it cancels in softmax -> only the *symmetrized* geometric
    product matters: ip = 0.5*(gp(sa,da) + gp(da,sa)) = sa^T Gsym da.
  * Work in a permuted Clifford basis (grade-ordered) so the vector
    components sit in contiguous slots 1..4.
  * mp2 columns are permuted into an L^T layout so the Cholesky factor comes
    out of the matmul directly; softplus on the diagonal via a degree-6
    polynomial (Horner on gpsimd).
  * metric @ v evaluated as L (L^T v) per edge batch-major on gpsimd.
  * cto_W is folded into the Gsym contraction: out += Wcto^T ip
    == (Gsym_half @ Wcto)^T P_half summed over both halves, so the ip
    tensor is never materialized (saves 2 matmuls + 1 eviction).

Performance structure (v3): engine-balanced to the CoreSim cost model.
  * Inputs arrive bf16 feature-major via the XBAR transposing DMA.
  * PSUM evictions spread across Pool (tensor_scalar, no access bubble),
    DVE and Act so no engine exceeds the PE's matmul time.
  * softmax normalization via a single tensor_tensor divide against a
    matmul-broadcast denominator (no reciprocal / re-broadcast chain).
  * metric math (softplus poly, L L^T v) runs batch-major fully on Pool
    with unrolled adds instead of DVE reductions.
  * output is written bf16 with a feature-major scatter DMA (no output
    transposes); host casts back to f32.
"""

import numpy as np
import ml_dtypes

import bass_rust as _bass_rust
import concourse.bass as bass
import concourse.mybir as mybir
from concourse.alu_op_type import AluOpType
from concourse.bass import ts
from concourse.tile import TileContext
from concourse.bass_utils import run_bass_kernel_spmd

F32 = mybir.dt.float32
FR = mybir.dt.float32r
BF = mybir.dt.bfloat16
AF = mybir.ActivationFunctionType
BF_NP = ml_dtypes.bfloat16

N_CORES = 8
B_FULL = 262144
D = 128
CH = 512          # edges per chunk
SUB = CH // 128   # 128-row subtiles per chunk

# grade-ordered Clifford basis permutation for Cl(4,0)
PERM = np.array([0, 1, 2, 4, 8, 3, 5, 6, 9, 10, 12, 7, 11, 13, 14, 15])

# ---- f32 consts layout (columns of the packed [128, NCOLS] array) ----
OFF_WI2 = 0
OFF_WMP1 = 128
OFF_WMP2 = 256
OFF_ID = 272      # f32 identity (used via fp32r bitcast for transposes)
OFF_BIAS = 400
# bias cols: 2 im1, 3 im2, 4 mp1, 5 q, 6 k, 7 v, 8 out+cto, 9 mp2pp(16),
#            10 msd(48: ntc bias at rows 16:32 and 48:64)
OFF_SPC = OFF_BIAS + 11   # 7 softplus polynomial coeff columns (c0..c6)
NCOLS = OFF_SPC + 7

# ---- bf16 consts layout (columns of the packed [128, NBCOLS] array) ----
BO_ID = 0         # bf16 identity [128,128]
BO_WNP = 128
BO_WEP = 256
BO_WQ = 384
BO_WK = 512
BO_WV = 640
BO_WOUT = 768
BO_EXP8 = 896     # [8 -> 128] head-broadcast lhsT
BO_WNTC = 1024    # 16
BO_GAW = 1040     # 128: gsym_p[:128] @ Wcto_p
BO_GBW = 1168     # 128: gsym_p[128:] @ Wcto_p
BO_BLK = 1296     # 8: per-head reduction lhsT
BO_O8 = 1304      # 8 cols of ones on 8 partitions (denominator broadcast)
NBCOLS = 1312


def _geo_table():
    N = 4
    BASIS = 16
    geo = np.zeros((BASIS, BASIS, BASIS), np.float64)
    for i in range(BASIS):
        bi = [(i >> (N - 1 - k)) & 1 for k in range(N)]
        for j in range(BASIS):
            bj = [(j >> (N - 1 - k)) & 1 for k in range(N)]
            sign = 1.0
            for k in range(N):
                if bj[k] and sum(bi[:k]) % 2 == 1:
                    sign = -sign
            geo[i, j, i ^ j] = sign
    return geo


def _pack_consts(w):
    """Build the [128, NCOLS] fp32 and [128, NBCOLS] bf16 constants arrays."""
    C = np.zeros((128, NCOLS), np.float32)
    CB = np.zeros((128, NBCOLS), np.float32)

    CB[:, BO_ID:BO_ID + 128] = np.eye(128, dtype=np.float32)
    # first two linear layers fused: relu(src@A + edge@B + b') with
    # A = np_W @ im1_W[:128], B = ep_W @ im1_W[128:]
    im1a = w["im1_W"][:128].astype(np.float64)
    im1b = w["im1_W"][128:].astype(np.float64)
    CB[:, BO_WNP:BO_WNP + 128] = (w["np_W"].astype(np.float64) @ im1a)
    CB[:, BO_WEP:BO_WEP + 128] = (w["ep_W"].astype(np.float64) @ im1b)
    CB[:, BO_WQ:BO_WQ + 128] = w["q_W"]
    CB[:, BO_WK:BO_WK + 128] = w["k_W"]
    CB[:, BO_WV:BO_WV + 128] = w["v_W"]
    CB[:, BO_WOUT:BO_WOUT + 128] = w["out_W"]

    C[:, OFF_WI2:OFF_WI2 + 128] = w["im2_W"]
    C[:, OFF_WMP1:OFF_WMP1 + 128] = w["mp1_W"]
    C[:, OFF_ID:OFF_ID + 128] = np.eye(128, dtype=np.float32)

    # mp2 in L^T ("Lt") layout: col j*4+i <- m[i,j] for i>=j, else zero
    W16 = np.zeros((128, 16), np.float32)
    b16 = np.zeros(16, np.float32)
    for j in range(4):
        for i in range(4):
            if i >= j:
                W16[:, j * 4 + i] = w["mp2_W"][:, i * 4 + j]
                b16[j * 4 + i] = w["mp2_b"][i * 4 + j]
    C[:, OFF_WMP2:OFF_WMP2 + 16] = W16

    ntc_Wp = w["ntc_W"][:, PERM]
    ntc_bp = w["ntc_b"][PERM]
    CB[:, BO_WNTC:BO_WNTC + 16] = ntc_Wp

    geo = _geo_table()
    gsym = 0.5 * (geo + np.swapaxes(geo, 0, 1))
    gsym_p = gsym[np.ix_(PERM, PERM, PERM)].reshape(256, 16)
    cto_p = w["cto_W"][PERM, :].astype(np.float64)
    CB[:, BO_GAW:BO_GAW + 128] = gsym_p[:128] @ cto_p
    CB[:, BO_GBW:BO_GBW + 128] = gsym_p[128:] @ cto_p

    # expand8[h, hd] = 1 if hd//16 == h  (lhsT for head-broadcast matmul)
    exp8 = np.zeros((8, 128), np.float32)
    for h in range(8):
        exp8[h, h * 16:(h + 1) * 16] = 1.0
    CB[:8, BO_EXP8:BO_EXP8 + 128] = exp8
    CB[:, BO_BLK:BO_BLK + 8] = exp8.T
    CB[:8, BO_O8:BO_O8 + 8] = 1.0

    C[:, OFF_BIAS + 2] = (w["np_b"].astype(np.float64) @ im1a
                          + w["ep_b"].astype(np.float64) @ im1b
                          + w["im1_b"])
    C[:, OFF_BIAS + 3] = w["im2_b"]
    C[:, OFF_BIAS + 4] = w["mp1_b"]
    C[:, OFF_BIAS + 5] = w["q_b"]
    C[:, OFF_BIAS + 6] = w["k_b"]
    C[:, OFF_BIAS + 7] = w["v_b"]
    C[:, OFF_BIAS + 8] = w["out_b"] + w["cto_b"]
    C[:16, OFF_BIAS + 9] = b16
    # bias for the merged msd tile [mt@0 | sct@32 | pad@48 | dct@64]
    C[32:48, OFF_BIAS + 10] = ntc_bp
    C[64:80, OFF_BIAS + 10] = ntc_bp

    # softplus-on-[-1,1] polynomial coefficients, replicated per partition
    xs = np.linspace(-1.0, 1.0, 20001)
    cheb = np.polynomial.chebyshev.Chebyshev.fit(xs, np.log1p(np.exp(xs)), 6)
    spc = cheb.convert(kind=np.polynomial.Polynomial).coef
    for k in range(7):
        C[:, OFF_SPC + k] = spc[k]
    return C, CB.astype(BF_NP)


def build_kernel(n_edges, stage=6, reps=1):
    """Build the per-core Bass program: n_edges rows -> n_edges rows.

    Software-pipelined: chunk c's serial tail (metric/P/output) is emitted
    around chunk c+1's head (loads/attention/MLP).
    """
    assert n_edges % CH == 0
    n_chunks = n_edges // CH

    nc = bass.Bass()
    src_d = nc.dram_tensor("src", [n_edges, D], BF, kind="ExternalInput")
    dst_d = nc.dram_tensor("dst", [n_edges, D], BF, kind="ExternalInput")
    edg_d = nc.dram_tensor("edg", [n_edges, D], BF, kind="ExternalInput")
    cst_d = nc.dram_tensor("consts", [128, NCOLS], F32, kind="ExternalInput")
    cbf_d = nc.dram_tensor("cbf", [128, NBCOLS], BF, kind="ExternalInput")
    out_d = nc.dram_tensor("out", [n_edges, D], BF, kind="ExternalOutput")

    with TileContext(nc) as tc, nc.allow_low_precision(
            reason="bf16/fp32r matmul operand rounding"), \
            nc.allow_non_contiguous_dma(
                reason="feature-major bf16 scatter store"):
        with (
            tc.tile_pool(name="cst", bufs=1) as cstp,
            tc.tile_pool(name="inn", bufs=3) as innp,
            tc.tile_pool(name="act", bufs=3) as actp,
            tc.tile_pool(name="big", bufs=3) as bigp,
            tc.tile_pool(name="sml", bufs=3) as smlp,
            tc.tile_pool(name="psm", bufs=3, space="PSUM") as psm,
            tc.tile_pool(name="pss", bufs=1, space="PSUM") as pss,
            tc.tile_pool(name="pnt", bufs=2, space="PSUM") as pntp,
            tc.tile_pool(name="ptp", bufs=1, space="PSUM") as ptp,
        ):
            cst = cstp.tile([128, NCOLS], F32)
            nc.sync.dma_start(out=cst[:], in_=cst_d[:])
            cbf = cstp.tile([128, NBCOLS], BF)
            nc.sync.dma_start(out=cbf[:], in_=cbf_d[:])

            # one-time rounding copy of interior fp32 weights to fp32r
            cstr = cstp.tile([128, OFF_BIAS], FR, tag="cstr")
            nc.vector.tensor_copy(cstr[:], cst[:, 0:OFF_BIAS])

            identb = cbf[:, BO_ID:BO_ID + 128]
            identr = cstr[:, OFF_ID:OFF_ID + 128]

            def WB(off, n=128, p=128):
                return cbf[:p, off:off + n]

            def WR(off, n=128, p=128):
                return cstr[:p, off:off + n]

            def bias(j, p=128, r0=0):
                return cst[r0:r0 + p, OFF_BIAS + j:OFF_BIAS + j + 1]

            def emit_head(c):
                row0 = c * CH
                st = {}
                fm = {}
                for name, dram in (("src", src_d), ("dst", dst_d),
                                   ("edg", edg_d)):
                    t = innp.tile([128, CH], BF, tag=f"in_{name}")
                    nc.sync.dma_start_transpose(
                        t[:], dram[row0:row0 + CH, :])
                    fm[name] = t

                # attention Q/K (independent of the MLP chain)
                pq = psm.tile([128, CH], F32, tag="mm")
                nc.tensor.matmul(pq[:], WB(BO_WQ), fm["src"][:], start=True,
                                 stop=True)
                pk = psm.tile([128, CH], F32, tag="mm")
                nc.tensor.matmul(pk[:], WB(BO_WK), fm["dst"][:], start=True,
                                 stop=True)
                ksb = smlp.tile([128, CH], BF, tag="ksb")
                nc.scalar.activation(ksb[:], pk[:], AF.Identity, bias=bias(6))
                prod = bigp.tile([128, CH], BF, tag="prod")
                nc.vector.scalar_tensor_tensor(
                    prod[:], pq[:], bias(5), ksb[:],
                    op0=AluOpType.add, op1=AluOpType.mult)

                # metric MLP layer 1 (np/ep fused into im1)
                ps1 = psm.tile([128, CH], F32, tag="mm")
                nc.tensor.matmul(ps1[:], WB(BO_WNP), fm["src"][:], start=True,
                                 stop=False)
                nc.tensor.matmul(ps1[:], WB(BO_WEP), fm["edg"][:], start=False,
                                 stop=True)
                i1 = actp.tile([128, CH], FR, tag="i1")
                nc.scalar.activation(i1[:], ps1[:], AF.Relu, bias=bias(2))

                pv = psm.tile([128, CH], F32, tag="mm")
                nc.tensor.matmul(pv[:], WB(BO_WV), fm["dst"][:], start=True,
                                 stop=True)
                vsb = smlp.tile([128, CH], BF, tag="vsb")
                nc.vector.tensor_scalar(vsb[:], pv[:], bias(7), 0.0,
                                        op0=AluOpType.add,
                                        op1=AluOpType.bypass)

                # head scores -> softmax numerator / denominator
                # (one PSUM bank: psc@0:8, den broadcast @32:40)
                pscd = pss.tile([40, CH], F32, tag="sm")
                nc.tensor.matmul(pscd[0:8, :], WB(BO_BLK, n=8), prod[:],
                                 start=True, stop=True)
                e = smlp.tile([8, CH], BF, tag="e")
                nc.scalar.activation(e[:], pscd[0:8, :], AF.Exp, scale=0.25)
                nc.tensor.matmul(pscd[32:40, :], WB(BO_O8, n=8, p=8), e[:],
                                 start=True, stop=True)
                wsb = smlp.tile([8, CH], BF, tag="w")
                nc.vector.tensor_tensor(wsb[:], e[:], pscd[32:40, :],
                                        op=AluOpType.divide)

                # metric MLP layers 2/3 (Pool evictions)
                ps2 = psm.tile([128, CH], F32, tag="mm")
                nc.tensor.matmul(ps2[:], WR(OFF_WI2), i1[:], start=True,
                                 stop=True)
                i2 = actp.tile([128, CH], FR, tag="i2")
                nc.gpsimd.tensor_scalar(i2[:], ps2[:], bias(3), 0.0,
                                        op0=AluOpType.add, op1=AluOpType.max)
                ps3 = psm.tile([128, CH], F32, tag="mm")
                nc.tensor.matmul(ps3[:], WR(OFF_WMP1), i2[:], start=True,
                                 stop=True)
                m1 = actp.tile([128, CH], FR, tag="m1")
                nc.gpsimd.tensor_scalar(m1[:], ps3[:], bias(4), 0.0,
                                        op0=AluOpType.add, op1=AluOpType.max)

                # attention combine
                pwr = psm.tile([128, CH], F32, tag="mm")
                nc.tensor.matmul(pwr[:], WB(BO_EXP8, p=8), wsb[:],
                                 start=True, stop=True)
                att = bigp.tile([128, CH], BF, tag="att")
                nc.vector.tensor_tensor(att[:], pwr[:], vsb[:],
                                        op=AluOpType.mult)
                st["att"] = att

                # mp2 + ntc heads share one PSUM tile:
                # [mt@0 | sct@32 | pad@48 | dct@64] (matmul PSUM writes need
                # 32-aligned partition bases; pad rows pre-zeroed once).
                pm96 = pntp.tile([80, CH], F32, tag="pm96")
                nc.tensor.matmul(pm96[0:16, :], WR(OFF_WMP2, n=16), m1[:],
                                 start=True, stop=True)
                nc.tensor.matmul(pm96[32:48, :], WB(BO_WNTC, n=16),
                                 fm["src"][:], start=True, stop=True)
                nc.tensor.matmul(pm96[64:80, :], WB(BO_WNTC, n=16),
                                 fm["dst"][:], start=True, stop=True)
                # packed msd tile [mt@0 | sct@32 | pad@48 | dct@64]
                # (SBUF engine writes: start 32 spans <=32, start 64 <=64)
                msd = smlp.tile([80, CH], FR, tag="msd")
                nc.scalar.activation(msd[0:16, :], pm96[0:16, :], AF.Tanh,
                                     bias=bias(9, p=16))
                nc.gpsimd.tensor_scalar(msd[32:48, :], pm96[32:48, :],
                                        bias(10, p=16, r0=32), 0.0,
                                        op0=AluOpType.add,
                                        op1=AluOpType.bypass)
                nc.gpsimd.tensor_scalar(msd[64:80, :], pm96[64:80, :],
                                        bias(10, p=16, r0=64), 0.0,
                                        op0=AluOpType.add,
                                        op1=AluOpType.bypass)
                st["msd"] = msd
                return st

            def emit_tail1(c, st):
                msd = st["msd"]
                bm = bigp.tile([128, SUB, 80], F32, tag="bm")
                for n in range(SUB):
                    pbm = ptp.tile([128, 80], FR, tag="tpo")
                    nc.tensor.transpose(pbm[:], msd[:, ts(n, 128)],
                                        identr[:80, :80])
                    nc.gpsimd.tensor_copy(bm[:, n, :], pbm[:].bitcast(F32))

                # softplus on the tanh-bounded diagonal via a degree-6
                # polynomial (Horner, gpsimd tensor_tensor only).
                dg = bm[:, :, 0:16:5]

                def spcb(k):
                    return cst[:, OFF_SPC + k:OFF_SPC + k + 1] \
                        .unsqueeze(1).broadcast_to((128, SUB, 4))

                u = bigp.tile([128, SUB, 4], F32, tag="spt")
                nc.gpsimd.tensor_tensor(u[:], dg, spcb(6), op=AluOpType.mult)
                for k in (5, 4, 3, 2, 1):
                    nc.gpsimd.tensor_tensor(u[:], u[:], spcb(k),
                                            op=AluOpType.add)
                    nc.gpsimd.tensor_tensor(u[:], u[:], dg,
                                            op=AluOpType.mult)
                nc.gpsimd.tensor_tensor(dg, u[:], spcb(0), op=AluOpType.add)

                # adaptive metric: v <- L (L^T v), in place, all on Pool
                # (unrolled adds along the 4-dim instead of DVE reductions)
                lt = bm[:, :, 0:16].rearrange("p n (j i) -> p n j i", j=4)
                for off in (32, 64):
                    v = bm[:, :, off + 1:off + 5]
                    tmp1 = bigp.tile([128, SUB, 4, 4], F32, tag="tmp1")
                    nc.gpsimd.tensor_tensor(
                        tmp1[:], lt,
                        v.unsqueeze(2).broadcast_to((128, SUB, 4, 4)),
                        op=AluOpType.mult)
                    t4 = bigp.tile([128, SUB, 4, 1], F32, tag="t4")
                    nc.gpsimd.tensor_tensor(t4[:], tmp1[:, :, :, 0:1],
                                            tmp1[:, :, :, 1:2],
                                            op=AluOpType.add)
                    nc.gpsimd.tensor_tensor(t4[:], t4[:], tmp1[:, :, :, 2:3],
                                            op=AluOpType.add)
                    nc.gpsimd.tensor_tensor(t4[:], t4[:], tmp1[:, :, :, 3:4],
                                            op=AluOpType.add)
                    tmp2 = bigp.tile([128, SUB, 4, 4], F32, tag="tmp1")
                    nc.gpsimd.tensor_tensor(
                        tmp2[:], lt,
                        t4[:].broadcast_to((128, SUB, 4, 4)),
                        op=AluOpType.mult)
                    tT = tmp2[:].transpose([0, 1, 3, 2])
                    u4 = bigp.tile([128, SUB, 4, 1], F32, tag="t4")
                    nc.gpsimd.tensor_tensor(u4[:], tT[:, :, :, 0:1],
                                            tT[:, :, :, 1:2],
                                            op=AluOpType.add)
                    nc.gpsimd.tensor_tensor(u4[:], u4[:], tT[:, :, :, 2:3],
                                            op=AluOpType.add)
                    nc.gpsimd.tensor_tensor(u4[:], u4[:], tT[:, :, :, 3:4],
                                            op=AluOpType.add)
                    nc.gpsimd.tensor_copy(v.unsqueeze(3), u4[:])

                # bf16 copies of full sa/da multivectors, then P = sa (x) da
                sab = bigp.tile([128, SUB, 16], BF, tag="sab")
                nc.gpsimd.tensor_copy(sab[:], bm[:, :, 32:48])
                dab = bigp.tile([128, SUB, 16], BF, tag="dab")
                nc.gpsimd.tensor_copy(dab[:], bm[:, :, 64:80])
                pP = bigp.tile([128, SUB, 256], BF, tag="pP")
                nc.gpsimd.tensor_tensor(
                    pP[:],
                    sab[:].unsqueeze(3).broadcast_to((128, SUB, 16, 16)),
                    dab[:].unsqueeze(2).broadcast_to((128, SUB, 16, 16)),
                    op=AluOpType.mult)
                st["pP"] = pP

            def emit_tail2(c, st):
                row0 = c * CH
                pP, att = st["pP"], st["att"]

                pfa = ptp.tile([128, CH], BF, tag="tpa")
                for n in range(SUB):
                    nc.tensor.transpose(pfa[:, ts(n, 128)], pP[:, n, 0:128],
                                        identb)
                Pa = bigp.tile([128, CH], BF, tag="Pa")
                nc.vector.tensor_copy(Pa[:], pfa[:])

                pfb = ptp.tile([128, CH], BF, tag="tpa")
                for n in range(SUB):
                    nc.tensor.transpose(pfb[:, ts(n, 128)], pP[:, n, 128:256],
                                        identb)
                Pb = bigp.tile([128, CH], BF, tag="Pb")
                nc.vector.tensor_copy(Pb[:], pfb[:])

                po = psm.tile([128, CH], F32, tag="mm")
                nc.tensor.matmul(po[:], WB(BO_WOUT), att[:], start=True,
                                 stop=False)
                nc.tensor.matmul(po[:], WB(BO_GAW), Pa[:], start=False,
                                 stop=False)
                nc.tensor.matmul(po[:], WB(BO_GBW), Pb[:], start=False,
                                 stop=True)
                of = bigp.tile([128, CH], BF, tag="of")
                nc.gpsimd.tensor_scalar(of[:], po[:], bias(8), 0.0,
                                        op0=AluOpType.add,
                                        op1=AluOpType.bypass)
                nc.sync.dma_start(
                    out=out_d[row0:row0 + CH, :].rearrange("b f -> f b"),
                    in_=of[:])

            prev = None
            prev_c = None
            for i in range(n_chunks * reps):
                c = i % n_chunks
                st = emit_head(c)
                emit_tail1(c, st)
                if prev is not None:
                    emit_tail2(prev_c, prev)
                prev, prev_c = st, c
            emit_tail2(prev_c, prev)

    # TRN2 allows at most one semaphore wait per engine instruction; split
    # multi-wait instructions through event semaphores (same pass bacc runs).
    _bass_rust.generate_event_semaphores(nc)
    return nc


_CACHE = {}


def _get_kernel(n_edges, stage=6):
    key = (n_edges, stage)
    if key not in _CACHE:
        _CACHE[key] = build_kernel(n_edges, stage)
    return _CACHE[key]


def kernel(**inputs):
    w = {k: np.asarray(v, np.float32) for k, v in inputs.items()
         if k not in ("src_nodes", "dst_nodes", "edge_features", "edge_indices")}
    src = np.ascontiguousarray(
        np.asarray(inputs["src_nodes"], np.float32).astype(BF_NP))
    dst = np.ascontiguousarray(
        np.asarray(inputs["dst_nodes"], np.float32).astype(BF_NP))
    edg = np.ascontiguousarray(
        np.asarray(inputs["edge_features"], np.float32).astype(BF_NP))

    B = src.shape[0]
    assert B % N_CORES == 0
    bc = B // N_CORES

    consts, cbf = _pack_consts(w)
    nc = _get_kernel(bc)

    in_maps = []
    for i in range(N_CORES):
        sl = slice(i * bc, (i + 1) * bc)
        in_maps.append({
            "src": src[sl], "dst": dst[sl], "edg": edg[sl],
            "consts": consts, "cbf": cbf,
        })
    res = run_bass_kernel_spmd(nc, in_maps, list(range(N_CORES)))
    return np.concatenate(
        [res.results[i]["out"].astype(np.float32) for i in range(N_CORES)],
        axis=0)


# revision 19
# speedup vs baseline: 1.3618x; 1.0716x over previous
"""Trainium2 Bass kernel for CliffordAdaptiveGraphAttention.

Math restructuring (validated numerically against the reference):
  * The "clifford enhancement" term `enh` is added uniformly to all 8 head
    scores, so it cancels in softmax -> only the *symmetrized* geometric
    product matters: ip = 0.5*(gp(sa,da) + gp(da,sa)) = sa^T Gsym da.
  * Work in a permuted Clifford basis (grade-ordered) so the vector
    components sit in contiguous slots 1..4.
  * mp2 columns are permuted into an L^T layout so the Cholesky factor comes
    out of the matmul directly; softplus on the diagonal via a degree-6
    polynomial (Horner on gpsimd).
  * metric @ v evaluated as L (L^T v) per edge batch-major on gpsimd.
  * cto_W is folded into the Gsym contraction: out += Wcto^T ip
    == (Gsym_half @ Wcto)^T P_half summed over both halves, so the ip
    tensor is never materialized (saves 2 matmuls + 1 eviction).

Performance structure (v3): engine-balanced to the CoreSim cost model.
  * Inputs arrive bf16 feature-major via the XBAR transposing DMA.
  * PSUM evictions spread across Pool (tensor_scalar, no access bubble),
    DVE and Act so no engine exceeds the PE's matmul time.
  * softmax normalization via a single tensor_tensor divide against a
    matmul-broadcast denominator (no reciprocal / re-broadcast chain).
  * metric math (softplus poly, L L^T v) runs batch-major fully on Pool
    with unrolled adds instead of DVE reductions.
  * output is written bf16 with a feature-major scatter DMA (no output
    transposes); host casts back to f32.
"""

import numpy as np
import ml_dtypes

import bass_rust as _bass_rust
import concourse.bass as bass
import concourse.mybir as mybir
from concourse.alu_op_type import AluOpType
from concourse.bass import ts
from concourse.tile import TileContext
from concourse.bass_utils import run_bass_kernel_spmd

F32 = mybir.dt.float32
FR = mybir.dt.float32r
BF = mybir.dt.bfloat16
AF = mybir.ActivationFunctionType
BF_NP = ml_dtypes.bfloat16

N_CORES = 8
B_FULL = 262144
D = 128
CH = 512          # edges per chunk
SUB = CH // 128   # 128-row subtiles per chunk

# grade-ordered Clifford basis permutation for Cl(4,0)
PERM = np.array([0, 1, 2, 4, 8, 3, 5, 6, 9, 10, 12, 7, 11, 13, 14, 15])

# ---- f32 consts layout (columns of the packed [128, NCOLS] array) ----
OFF_WI2 = 0
OFF_WMP1 = 128
OFF_WMP2 = 256
OFF_ID = 272      # f32 identity (used via fp32r bitcast for transposes)
OFF_BIAS = 400
# bias cols: 2 im1, 3 im2, 4 mp1, 5 q, 6 k, 7 v, 8 out+cto, 9 mp2pp(16),
#            10 msd(48: ntc bias at rows 16:32 and 48:64)
OFF_SPC = OFF_BIAS + 11   # 7 softplus polynomial coeff columns (c0..c6)
NCOLS = OFF_SPC + 7

# ---- bf16 consts layout (columns of the packed [128, NBCOLS] array) ----
BO_ID = 0         # bf16 identity [128,128]
BO_WNP = 128
BO_WEP = 256
BO_WQ = 384
BO_WK = 512
BO_WV = 640
BO_WOUT = 768
BO_EXP8 = 896     # [8 -> 128] head-broadcast lhsT
BO_WNTC = 1024    # 16
BO_GAW = 1040     # 128: gsym_p[:128] @ Wcto_p
BO_GBW = 1168     # 128: gsym_p[128:] @ Wcto_p
BO_BLK = 1296     # 8: per-head reduction lhsT
BO_O8 = 1304      # 8 cols of ones on 8 partitions (denominator broadcast)
NBCOLS = 1312


def _geo_table():
    N = 4
    BASIS = 16
    geo = np.zeros((BASIS, BASIS, BASIS), np.float64)
    for i in range(BASIS):
        bi = [(i >> (N - 1 - k)) & 1 for k in range(N)]
        for j in range(BASIS):
            bj = [(j >> (N - 1 - k)) & 1 for k in range(N)]
            sign = 1.0
            for k in range(N):
                if bj[k] and sum(bi[:k]) % 2 == 1:
                    sign = -sign
            geo[i, j, i ^ j] = sign
    return geo


def _pack_consts(w):
    """Build the [128, NCOLS] fp32 and [128, NBCOLS] bf16 constants arrays."""
    C = np.zeros((128, NCOLS), np.float32)
    CB = np.zeros((128, NBCOLS), np.float32)

    CB[:, BO_ID:BO_ID + 128] = np.eye(128, dtype=np.float32)
    # first two linear layers fused: relu(src@A + edge@B + b') with
    # A = np_W @ im1_W[:128], B = ep_W @ im1_W[128:]
    im1a = w["im1_W"][:128].astype(np.float64)
    im1b = w["im1_W"][128:].astype(np.float64)
    CB[:, BO_WNP:BO_WNP + 128] = (w["np_W"].astype(np.float64) @ im1a)
    CB[:, BO_WEP:BO_WEP + 128] = (w["ep_W"].astype(np.float64) @ im1b)
    CB[:, BO_WQ:BO_WQ + 128] = w["q_W"]
    CB[:, BO_WK:BO_WK + 128] = w["k_W"]
    CB[:, BO_WV:BO_WV + 128] = w["v_W"]
    CB[:, BO_WOUT:BO_WOUT + 128] = w["out_W"]

    C[:, OFF_WI2:OFF_WI2 + 128] = w["im2_W"]
    C[:, OFF_WMP1:OFF_WMP1 + 128] = w["mp1_W"]
    C[:, OFF_ID:OFF_ID + 128] = np.eye(128, dtype=np.float32)

    # mp2 in L^T ("Lt") layout: col j*4+i <- m[i,j] for i>=j, else zero
    W16 = np.zeros((128, 16), np.float32)
    b16 = np.zeros(16, np.float32)
    for j in range(4):
        for i in range(4):
            if i >= j:
                W16[:, j * 4 + i] = w["mp2_W"][:, i * 4 + j]
                b16[j * 4 + i] = w["mp2_b"][i * 4 + j]
    C[:, OFF_WMP2:OFF_WMP2 + 16] = W16

    ntc_Wp = w["ntc_W"][:, PERM]
    ntc_bp = w["ntc_b"][PERM]
    CB[:, BO_WNTC:BO_WNTC + 16] = ntc_Wp

    geo = _geo_table()
    gsym = 0.5 * (geo + np.swapaxes(geo, 0, 1))
    gsym_p = gsym[np.ix_(PERM, PERM, PERM)].reshape(256, 16)
    cto_p = w["cto_W"][PERM, :].astype(np.float64)
    CB[:, BO_GAW:BO_GAW + 128] = gsym_p[:128] @ cto_p
    CB[:, BO_GBW:BO_GBW + 128] = gsym_p[128:] @ cto_p

    # expand8[h, hd] = 1 if hd//16 == h  (lhsT for head-broadcast matmul)
    exp8 = np.zeros((8, 128), np.float32)
    for h in range(8):
        exp8[h, h * 16:(h + 1) * 16] = 1.0
    CB[:8, BO_EXP8:BO_EXP8 + 128] = exp8
    CB[:, BO_BLK:BO_BLK + 8] = exp8.T
    CB[:8, BO_O8:BO_O8 + 8] = 1.0

    C[:, OFF_BIAS + 2] = (w["np_b"].astype(np.float64) @ im1a
                          + w["ep_b"].astype(np.float64) @ im1b
                          + w["im1_b"])
    C[:, OFF_BIAS + 3] = w["im2_b"]
    C[:, OFF_BIAS + 4] = w["mp1_b"]
    C[:, OFF_BIAS + 5] = w["q_b"]
    C[:, OFF_BIAS + 6] = w["k_b"]
    C[:, OFF_BIAS + 7] = w["v_b"]
    C[:, OFF_BIAS + 8] = w["out_b"] + w["cto_b"]
    C[:16, OFF_BIAS + 9] = b16
    # bias for the merged msd tile [mt@0 | sct@32 | pad@48 | dct@64]
    C[32:48, OFF_BIAS + 10] = ntc_bp
    C[64:80, OFF_BIAS + 10] = ntc_bp

    # softplus-on-[-1,1] polynomial coefficients, replicated per partition
    xs = np.linspace(-1.0, 1.0, 20001)
    cheb = np.polynomial.chebyshev.Chebyshev.fit(xs, np.log1p(np.exp(xs)), 6)
    spc = cheb.convert(kind=np.polynomial.Polynomial).coef
    for k in range(7):
        C[:, OFF_SPC + k] = spc[k]
    return C, CB.astype(BF_NP)


def build_kernel(n_edges, stage=6, reps=1):
    """Build the per-core Bass program: n_edges rows -> n_edges rows.

    Software-pipelined: chunk c's serial tail (metric/P/output) is emitted
    around chunk c+1's head (loads/attention/MLP).
    """
    assert n_edges % CH == 0
    n_chunks = n_edges // CH

    nc = bass.Bass()
    src_d = nc.dram_tensor("src", [n_edges, D], BF, kind="ExternalInput")
    dst_d = nc.dram_tensor("dst", [n_edges, D], BF, kind="ExternalInput")
    edg_d = nc.dram_tensor("edg", [n_edges, D], BF, kind="ExternalInput")
    cst_d = nc.dram_tensor("consts", [128, NCOLS], F32, kind="ExternalInput")
    cbf_d = nc.dram_tensor("cbf", [128, NBCOLS], BF, kind="ExternalInput")
    out_d = nc.dram_tensor("out", [n_edges, D], BF, kind="ExternalOutput")

    with TileContext(nc) as tc, nc.allow_low_precision(
            reason="bf16/fp32r matmul operand rounding"), \
            nc.allow_non_contiguous_dma(
                reason="feature-major bf16 scatter store"):
        with (
            tc.tile_pool(name="cst", bufs=1) as cstp,
            tc.tile_pool(name="inn", bufs=3) as innp,
            tc.tile_pool(name="act", bufs=3) as actp,
            tc.tile_pool(name="big", bufs=3) as bigp,
            tc.tile_pool(name="sml", bufs=3) as smlp,
            tc.tile_pool(name="psm", bufs=3, space="PSUM") as psm,
            tc.tile_pool(name="pss", bufs=1, space="PSUM") as pss,
            tc.tile_pool(name="pnt", bufs=2, space="PSUM") as pntp,
            tc.tile_pool(name="ptp", bufs=1, space="PSUM") as ptp,
        ):
            cst = cstp.tile([128, NCOLS], F32)
            nc.sync.dma_start(out=cst[:], in_=cst_d[:])
            cbf = cstp.tile([128, NBCOLS], BF)
            nc.sync.dma_start(out=cbf[:], in_=cbf_d[:])

            # one-time rounding copy of interior fp32 weights to fp32r
            cstr = cstp.tile([128, OFF_BIAS], FR, tag="cstr")
            nc.vector.tensor_copy(cstr[:], cst[:, 0:OFF_BIAS])

            identb = cbf[:, BO_ID:BO_ID + 128]
            identr = cstr[:, OFF_ID:OFF_ID + 128]

            def WB(off, n=128, p=128):
                return cbf[:p, off:off + n]

            def WR(off, n=128, p=128):
                return cstr[:p, off:off + n]

            def bias(j, p=128, r0=0):
                return cst[r0:r0 + p, OFF_BIAS + j:OFF_BIAS + j + 1]

            def emit_head(c):
                row0 = c * CH
                st = {}
                fm = {}
                for name, dram in (("src", src_d), ("dst", dst_d),
                                   ("edg", edg_d)):
                    t = innp.tile([128, CH], BF, tag=f"in_{name}")
                    nc.sync.dma_start_transpose(
                        t[:], dram[row0:row0 + CH, :])
                    fm[name] = t

                # attention Q/K (independent of the MLP chain)
                pq = psm.tile([128, CH], F32, tag="mm")
                nc.tensor.matmul(pq[:], WB(BO_WQ), fm["src"][:], start=True,
                                 stop=True)
                pk = psm.tile([128, CH], F32, tag="mm")
                nc.tensor.matmul(pk[:], WB(BO_WK), fm["dst"][:], start=True,
                                 stop=True)
                ksb = smlp.tile([128, CH], BF, tag="ksb")
                nc.scalar.activation(ksb[:], pk[:], AF.Identity, bias=bias(6))
                prod = bigp.tile([128, CH], BF, tag="prod")
                nc.vector.scalar_tensor_tensor(
                    prod[:], pq[:], bias(5), ksb[:],
                    op0=AluOpType.add, op1=AluOpType.mult)

                # metric MLP layer 1 (np/ep fused into im1)
                ps1 = psm.tile([128, CH], F32, tag="mm")
                nc.tensor.matmul(ps1[:], WB(BO_WNP), fm["src"][:], start=True,
                                 stop=False)
                nc.tensor.matmul(ps1[:], WB(BO_WEP), fm["edg"][:], start=False,
                                 stop=True)
                i1 = actp.tile([128, CH], FR, tag="i1")
                nc.scalar.activation(i1[:], ps1[:], AF.Relu, bias=bias(2))

                pv = psm.tile([128, CH], F32, tag="mm")
                nc.tensor.matmul(pv[:], WB(BO_WV), fm["dst"][:], start=True,
                                 stop=True)
                vsb = smlp.tile([128, CH], BF, tag="vsb")
                nc.vector.tensor_scalar(vsb[:], pv[:], bias(7), 0.0,
                                        op0=AluOpType.add,
                                        op1=AluOpType.bypass)

                # head scores -> softmax numerator / denominator
                # (one PSUM bank: psc@0:8, den broadcast @32:40)
                pscd = pss.tile([40, CH], F32, tag="sm")
                nc.tensor.matmul(pscd[0:8, :], WB(BO_BLK, n=8), prod[:],
                                 start=True, stop=True)
                e = smlp.tile([8, CH], BF, tag="e")
                nc.scalar.activation(e[:], pscd[0:8, :], AF.Exp, scale=0.25)
                nc.tensor.matmul(pscd[32:40, :], WB(BO_O8, n=8, p=8), e[:],
                                 start=True, stop=True)
                wsb = smlp.tile([8, CH], BF, tag="w")
                nc.vector.tensor_tensor(wsb[:], e[:], pscd[32:40, :],
                                        op=AluOpType.divide)

                # metric MLP layers 2/3 (Pool evictions)
                ps2 = psm.tile([128, CH], F32, tag="mm")
                nc.tensor.matmul(ps2[:], WR(OFF_WI2), i1[:], start=True,
                                 stop=True)
                i2 = actp.tile([128, CH], FR, tag="i2")
                nc.gpsimd.tensor_scalar(i2[:], ps2[:], bias(3), 0.0,
                                        op0=AluOpType.add, op1=AluOpType.max)
                ps3 = psm.tile([128, CH], F32, tag="mm")
                nc.tensor.matmul(ps3[:], WR(OFF_WMP1), i2[:], start=True,
                                 stop=True)
                m1 = actp.tile([128, CH], FR, tag="m1")
                nc.gpsimd.tensor_scalar(m1[:], ps3[:], bias(4), 0.0,
                                        op0=AluOpType.add, op1=AluOpType.max)

                # attention combine
                pwr = psm.tile([128, CH], F32, tag="mm")
                nc.tensor.matmul(pwr[:], WB(BO_EXP8, p=8), wsb[:],
                                 start=True, stop=True)
                att = bigp.tile([128, CH], BF, tag="att")
                nc.vector.tensor_tensor(att[:], pwr[:], vsb[:],
                                        op=AluOpType.mult)
                st["att"] = att

                # mp2 + ntc heads share one PSUM tile:
                # [mt@0 | sct@32 | pad@48 | dct@64] (matmul PSUM writes need
                # 32-aligned partition bases; pad rows pre-zeroed once).
                pm96 = pntp.tile([80, CH], F32, tag="pm96")
                nc.tensor.matmul(pm96[0:16, :], WR(OFF_WMP2, n=16), m1[:],
                                 start=True, stop=True)
                nc.tensor.matmul(pm96[32:48, :], WB(BO_WNTC, n=16),
                                 fm["src"][:], start=True, stop=True)
                nc.tensor.matmul(pm96[64:80, :], WB(BO_WNTC, n=16),
                                 fm["dst"][:], start=True, stop=True)
                # packed msd tile [mt@0 | sct@32 | pad@48 | dct@64]
                # (SBUF engine writes: start 32 spans <=32, start 64 <=64)
                msd = smlp.tile([80, CH], FR, tag="msd")
                nc.scalar.activation(msd[0:16, :], pm96[0:16, :], AF.Tanh,
                                     bias=bias(9, p=16))
                nc.gpsimd.tensor_scalar(msd[32:48, :], pm96[32:48, :],
                                        bias(10, p=16, r0=32), 0.0,
                                        op0=AluOpType.add,
                                        op1=AluOpType.bypass)
                nc.gpsimd.tensor_scalar(msd[64:80, :], pm96[64:80, :],
                                        bias(10, p=16, r0=64), 0.0,
                                        op0=AluOpType.add,
                                        op1=AluOpType.bypass)
                st["msd"] = msd
                return st

            def emit_tail1(c, st):
                msd = st["msd"]
                bm = bigp.tile([128, SUB, 80], F32, tag="bm")
                pbm = ptp.tile([128, SUB, 80], FR, tag="tpo")
                for n in range(SUB):
                    nc.tensor.transpose(pbm[:, n, :], msd[:, ts(n, 128)],
                                        identr[:80, :80])
                nc.gpsimd.tensor_copy(bm[:], pbm[:].bitcast(F32))

                # softplus on the tanh-bounded diagonal via a degree-6
                # polynomial (Horner, gpsimd tensor_tensor only).
                dg = bm[:, :, 0:16:5]

                def spcb(k):
                    return cst[:, OFF_SPC + k:OFF_SPC + k + 1] \
                        .unsqueeze(1).broadcast_to((128, SUB, 4))

                u = bigp.tile([128, SUB, 4], F32, tag="spt")
                nc.gpsimd.tensor_tensor(u[:], dg, spcb(6), op=AluOpType.mult)
                for k in (5, 4, 3, 2, 1):
                    nc.gpsimd.tensor_tensor(u[:], u[:], spcb(k),
                                            op=AluOpType.add)
                    nc.gpsimd.tensor_tensor(u[:], u[:], dg,
                                            op=AluOpType.mult)
                nc.gpsimd.tensor_tensor(dg, u[:], spcb(0), op=AluOpType.add)

                # adaptive metric: v <- L (L^T v), in place, all on Pool
                # (unrolled adds along the 4-dim instead of DVE reductions)
                lt = bm[:, :, 0:16].rearrange("p n (j i) -> p n j i", j=4)
                for off in (32, 64):
                    v = bm[:, :, off + 1:off + 5]
                    tmp1 = bigp.tile([128, SUB, 4, 4], F32, tag="tmp1")
                    nc.gpsimd.tensor_tensor(
                        tmp1[:], lt,
                        v.unsqueeze(2).broadcast_to((128, SUB, 4, 4)),
                        op=AluOpType.mult)
                    t4 = bigp.tile([128, SUB, 4, 1], F32, tag="t4")
                    nc.gpsimd.tensor_tensor(t4[:], tmp1[:, :, :, 0:1],
                                            tmp1[:, :, :, 1:2],
                                            op=AluOpType.add)
                    nc.gpsimd.tensor_tensor(t4[:], t4[:], tmp1[:, :, :, 2:3],
                                            op=AluOpType.add)
                    nc.gpsimd.tensor_tensor(t4[:], t4[:], tmp1[:, :, :, 3:4],
                                            op=AluOpType.add)
                    tmp2 = bigp.tile([128, SUB, 4, 4], F32, tag="tmp1")
                    nc.gpsimd.tensor_tensor(
                        tmp2[:], lt,
                        t4[:].broadcast_to((128, SUB, 4, 4)),
                        op=AluOpType.mult)
                    tT = tmp2[:].transpose([0, 1, 3, 2])
                    u4 = bigp.tile([128, SUB, 4, 1], F32, tag="t4")
                    nc.gpsimd.tensor_tensor(u4[:], tT[:, :, :, 0:1],
                                            tT[:, :, :, 1:2],
                                            op=AluOpType.add)
                    nc.gpsimd.tensor_tensor(u4[:], u4[:], tT[:, :, :, 2:3],
                                            op=AluOpType.add)
                    nc.gpsimd.tensor_tensor(u4[:], u4[:], tT[:, :, :, 3:4],
                                            op=AluOpType.add)
                    nc.gpsimd.tensor_copy(v.unsqueeze(3), u4[:])

                # bf16 copies of full sa/da multivectors, then P = sa (x) da
                sab = bigp.tile([128, SUB, 16], BF, tag="sab")
                nc.gpsimd.tensor_copy(sab[:], bm[:, :, 32:48])
                dab = bigp.tile([128, SUB, 16], BF, tag="dab")
                nc.gpsimd.tensor_copy(dab[:], bm[:, :, 64:80])
                pP = bigp.tile([128, SUB, 256], BF, tag="pP")
                nc.gpsimd.tensor_tensor(
                    pP[:],
                    sab[:].unsqueeze(3).broadcast_to((128, SUB, 16, 16)),
                    dab[:].unsqueeze(2).broadcast_to((128, SUB, 16, 16)),
                    op=AluOpType.mult)
                st["pP"] = pP

            def emit_tail2(c, st):
                row0 = c * CH
                pP, att = st["pP"], st["att"]

                pf = ptp.tile([128, 2, CH], BF, tag="tpa")
                for n in range(SUB):
                    nc.tensor.transpose(pf[:, 0, ts(n, 128)], pP[:, n, 0:128],
                                        identb)
                    nc.tensor.transpose(pf[:, 1, ts(n, 128)],
                                        pP[:, n, 128:256], identb)
                Pa = bigp.tile([128, CH], BF, tag="Pa")
                nc.vector.tensor_copy(Pa[:], pf[:, 0, :])
                Pb = bigp.tile([128, CH], BF, tag="Pb")
                nc.vector.tensor_copy(Pb[:], pf[:, 1, :])

                po = psm.tile([128, CH], F32, tag="mm")
                nc.tensor.matmul(po[:], WB(BO_WOUT), att[:], start=True,
                                 stop=False)
                nc.tensor.matmul(po[:], WB(BO_GAW), Pa[:], start=False,
                                 stop=False)
                nc.tensor.matmul(po[:], WB(BO_GBW), Pb[:], start=False,
                                 stop=True)
                of = bigp.tile([128, CH], BF, tag="of")
                nc.gpsimd.tensor_scalar(of[:], po[:], bias(8), 0.0,
                                        op0=AluOpType.add,
                                        op1=AluOpType.bypass)
                nc.sync.dma_start(
                    out=out_d[row0:row0 + CH, :].rearrange("b f -> f b"),
                    in_=of[:])

            # 3-deep software pipeline: head(c) | tail1(c-1) | tail2(c-2)
            total = n_chunks * reps
            sts = {}
            for i in range(total + 2):
                if i < total:
                    sts[i] = emit_head(i % n_chunks)
                if 1 <= i < total + 1:
                    emit_tail1((i - 1) % n_chunks, sts[i - 1])
                if i >= 2:
                    emit_tail2((i - 2) % n_chunks, sts.pop(i - 2))

    # TRN2 allows at most one semaphore wait per engine instruction; split
    # multi-wait instructions through event semaphores (same pass bacc runs).
    _bass_rust.generate_event_semaphores(nc)
    return nc


_CACHE = {}


def _get_kernel(n_edges, stage=6):
    key = (n_edges, stage)
    if key not in _CACHE:
        _CACHE[key] = build_kernel(n_edges, stage)
    return _CACHE[key]


def kernel(**inputs):
    w = {k: np.asarray(v, np.float32) for k, v in inputs.items()
         if k not in ("src_nodes", "dst_nodes", "edge_features", "edge_indices")}
    src = np.ascontiguousarray(
        np.asarray(inputs["src_nodes"], np.float32).astype(BF_NP))
    dst = np.ascontiguousarray(
        np.asarray(inputs["dst_nodes"], np.float32).astype(BF_NP))
    edg = np.ascontiguousarray(
        np.asarray(inputs["edge_features"], np.float32).astype(BF_NP))

    B = src.shape[0]
    assert B % N_CORES == 0
    bc = B // N_CORES

    consts, cbf = _pack_consts(w)
    nc = _get_kernel(bc)

    in_maps = []
    for i in range(N_CORES):
        sl = slice(i * bc, (i + 1) * bc)
        in_maps.append({
            "src": src[sl], "dst": dst[sl], "edg": edg[sl],
            "consts": consts, "cbf": cbf,
        })
    res = run_bass_kernel_spmd(nc, in_maps, list(range(N_CORES)))
    return np.concatenate(
        [res.results[i]["out"].astype(np.float32) for i in range(N_CORES)],
        axis=0)
